# revision 11
# baseline (speedup 1.0000x reference)
"""Trainium2 Bass kernel for causal+padded multi-head attention.

Problem: B=2, N=2048, D=1024, H=16 heads (DK=64), fp32 I/O.
  out = softmax(mask(x Wq^T (x Wk^T)^T) / sqrt(DK)) (x Wv^T) Wout^T + b_out

Sharding (8 cores): core c handles batch b=c//4 and heads [4*(c%4), 4*(c%4)+4).
Each core computes a partial output [N, D] (its 4 heads' contribution through
the output projection, bf16); the host sums the 4 partials per batch in fp32
and adds b_out.

Schedule (v2, from the 156us baseline's trace):
  - PE p-state: TRN2 throttles the PE clock to 1.2/0.65 GHz after idle and
    only reaches 2.4 GHz after ~3us of continuous execution.  A chain of
    warm-up matmuls on a memset tile runs during the DMA head so real work
    starts at full clock, and the schedule keeps the PE dense to stay there.
  - Input DMAs are split per-128-row chunk (wq/wk/wv per e, xt per (e, qtile))
    and interleaved across BOTH hardware queues (SP + ACT) so the first
    projection matmul can start ~1us after the queues spin up instead of
    after the full weight load.
  - Attention units (pair, qtile) riffle their OWN PV matmuls into the S^T
    stream at a small lag (the PSUM ctx banks of the previous unit are
    released by its normalize, which is emitted just before this unit).
    Unit order ends on a 4-chunk qt=0 unit so the tail chain after the last
    exp is short.
  - Normalize is split into phases: the DVE transpose/reciprocal dance at the
    unit boundary, the GpSimd partition-broadcast right after, and the DVE
    multiplies flushed 2 chunks into the next unit's stream -- so the 1us
    broadcast latency never stalls the in-order DVE queue in front of the
    round evacuations that pace the PE.
  - The LAST unit's normalize runs in 256-column halves with the final
    out-projection rounds interleaved, and the final out DMAs are split per
    512-column half and alternated across both queues.

Known dead ends (measured): fp8 anywhere gives 4-8e-2 rel err (gate 1e-2);
reciprocal_approx_fast (custom DVE uop) returns garbage/crashes the exec unit
under this runtime; exp(-ln(den)) on ScalarE forces an ACT table swap per
call (~38us total); SBUF->SBUF DMA cannot cross partitions, and DRAM-bounce
transposes race (DMA queue issue is async, Tile does not serialize the DRAM
RAW).
"""

import math
import os

import numpy as np

B, N, D, H = 2, 2048, 1024, 16
DK = D // H  # 64
NCORES = 8
HEADS_PER_CORE = 4
QTILE = 512
KBLK = 128
NEG = -30000.0
NEGB = -3750.0  # pad bias applied after the 0.125 scale inside exp
SCALE = 1.0 / math.sqrt(float(DK))  # 0.125
RIFFLE_LAG = 3  # PV chunk j rides the same unit's S^T stream after chunk j+LAG
NWARM = 8  # PE clock warm-up matmuls during the DMA head

# Set by run() when tracing is enabled (test.py reads this).
LAST_RESULTS = None


def _build_program(kb_max: int, jpad_min: int):
    import concourse.tile as tile
    from concourse import bacc, mybir

    F32 = mybir.dt.float32
    BF16 = mybir.dt.bfloat16
    EXP = mybir.ActivationFunctionType.Exp
    ADD = mybir.AluOpType.add

    nc = bacc.Bacc(None)

    xt_d = nc.dram_tensor("xt", [D, N], BF16, kind="ExternalInput")
    wq_d = nc.dram_tensor("wq", [D, 256], BF16, kind="ExternalInput")
    wk_d = nc.dram_tensor("wk", [D, 256], BF16, kind="ExternalInput")
    wv_d = nc.dram_tensor("wv", [D, 256], BF16, kind="ExternalInput")
    wout_d = nc.dram_tensor("wout", [256, D], BF16, kind="ExternalInput")
    padb_d = nc.dram_tensor("padbias", [128, 16], F32, kind="ExternalInput")
    trineg_d = nc.dram_tensor("trineg", [128, 896], BF16, kind="ExternalInput")
    out_d = nc.dram_tensor("out", [N, D], BF16, kind="ExternalOutput")

    NB = N // KBLK  # 16 key/row blocks
    NQT = N // QTILE  # 4 q tiles

    with tile.TileContext(nc) as tc:
        with (
            tc.tile_pool(name="w", bufs=1) as w_pool,
            tc.tile_pool(name="big", bufs=1) as big_pool,
            tc.tile_pool(name="work", bufs=4) as work_pool,
            tc.tile_pool(name="osb", bufs=3) as osb_pool,
            tc.tile_pool(name="xt", bufs=1) as xt_pool,
            tc.tile_pool(name="pt", bufs=10) as pt_pool,
            tc.tile_pool(name="ps_st", bufs=2, space="PSUM") as ps_st,
            tc.tile_pool(name="ps_b", bufs=2, space="PSUM") as ps_b,
            tc.tile_pool(name="ps_ctx", bufs=1, space="PSUM") as ps_ctx,
        ):
            # ---- SBUF tiles ------------------------------------------------
            # weights/x split in e-halves: [128, 4 rows-of-128, cols].  One
            # dma_start costs ~0.7us of ISSUING-ENGINE time (and blocks on
            # ring backlog), so transfers are few and large; halves keep the
            # first matmul's dependency at 256KB instead of 512KB.
            wqh = [w_pool.tile([128, 4, 256], BF16, tag=f"wqh{h}", name=f"wqh{h}") for h in range(2)]
            wkh = [w_pool.tile([128, 4, 256], BF16, tag=f"wkh{h}", name=f"wkh{h}") for h in range(2)]
            wvh = [w_pool.tile([128, 4, 256], BF16, tag=f"wvh{h}", name=f"wvh{h}") for h in range(2)]
            wo2 = [w_pool.tile([128, D], BF16, tag=f"wo{c}", name=f"wo{c}") for c in range(2)]
            padb_t = w_pool.tile([128, 16], F32, tag="padb", name="padb")
            trineg_t = w_pool.tile([128, 896], BF16, tag="trineg", name="trineg")
            XCH = [2, 4, 4, 4]  # rows-of-128 per xt tile (c0 finer: earlier start)
            xth = [[xt_pool.tile([128, XCH[c], 512], BF16, tag=f"xt{c}_{h}",
                                 name=f"xt{c}_{h}")
                    for h in range(8 // XCH[c])] for c in range(NQT)]

            def wq8(e):
                return wqh[e // 4][:, e % 4, :]

            def wk8(e):
                return wkh[e // 4][:, e % 4, :]

            def wv8(e):
                return wvh[e // 4][:, e % 4, :]

            def xt(e, c):
                ch = XCH[c]
                return xth[c][e // ch][:, e % ch, :]

            # PE clock warm-up: a dense chain of matmuls on a memset tile
            # runs during the DMA head so the p-state ramp (0.65 -> 1.2 ->
            # 2.4 GHz, evaluated in ~4us HAM windows) finishes before the
            # first real matmul.
            warmw = w_pool.tile([128, 512], BF16, tag="warmw", name="warmw")
            nc.vector.memset(warmw[:], 0.03)
            wps = ps_b.tile([128, 512], F32, tag="b", name="b")
            for _ in range(NWARM):
                nc.tensor.matmul(wps[:], warmw[:, 0:128], warmw[:],
                                 start=True, stop=True)

            # ---- input DMAs ------------------------------------------------
            # sync queue carries the bulk; the scalar (ACT) queue gets ONLY
            # the three transfers needed before the first exp, then the warm
            # exp -- everything after would delay the exp stream (each
            # dma_start blocks the engine for max(0.7us, ring backlog)).
            def _ld(q, dst, dram, r0, r1, cols):
                q.dma_start(
                    dst[:],
                    dram[r0:r1, :].rearrange(
                        "(e p) m -> p e m", p=128
                    ) if cols is None else
                    dram[r0:r1, cols[0]:cols[1]].rearrange(
                        "(e p) m -> p e m", p=128
                    ),
                )

            _ld(nc.sync, wqh[0], wq_d, 0, 512, None)
            _ld(nc.scalar, wqh[1], wq_d, 512, 1024, None)
            for i in range(2):  # xt c0 quarters: sync 0-1, scalar 2-3
                _ld(nc.sync, xth[0][i], xt_d, i * 256, (i + 1) * 256, (0, 512))
            for i in range(2, 4):
                _ld(nc.scalar, xth[0][i], xt_d, i * 256, (i + 1) * 256, (0, 512))
            _ld(nc.sync, wkh[0], wk_d, 0, 512, None)
            _ld(nc.scalar, wkh[1], wk_d, 512, 1024, None)
            nc.sync.dma_start(trineg_t[:], trineg_d[:])

            # warm the ACT exp table now: the scalar engine has issued its 4
            # DMAs; the ~1.5us table load overlaps their transfers
            warm = work_pool.tile([1, 8], F32, tag="warm", name="warm")
            nc.vector.memset(warm[:], 1.0)
            nc.scalar.activation(warm[:], warm[:], EXP)

            _ld(nc.sync, wvh[0], wv_d, 0, 512, None)
            _ld(nc.scalar, wvh[1], wv_d, 512, 1024, None)
            for c in range(1, NQT):
                _ld(nc.sync, xth[c][0], xt_d, 0, 512, (c * 512, (c + 1) * 512))
                _ld(nc.scalar if c < 3 else nc.sync,
                    xth[c][1], xt_d, 512, 1024, (c * 512, (c + 1) * 512))
            nc.sync.dma_start(wo2[0][:], wout_d[0:128, :])
            nc.sync.dma_start(wo2[1][:], wout_d[128:256, :])
            nc.sync.dma_start(padb_t[:], padb_d[:])

            # bridge matmuls: after the fixed warm chain, keep the PE ticking
            # on each input tile AS IT LANDS (outputs never read) so there is
            # no idle gap -- and no HAM re-throttle -- between the warm-up
            # and the first projection round
            for brt in (wqh[0], wqh[1], xth[0][0], xth[0][2], xth[0][1],
                        xth[0][3], wkh[0], wkh[1]):
                nc.tensor.matmul(wps[:], brt[:].rearrange("p a m -> p (a m)")[:, 0:128],
                                 warmw[:], start=True, stop=True)

            # V' tile: [keys 128, key-block, head 4, 65]; col 64 <- ones so
            # P@V' also yields the softmax denominator on ctx row 64.
            v4 = big_pool.tile([128, kb_max, 4, 65], BF16, tag="v4", name="v4")
            nc.gpsimd.memset(v4[:, :, :, 64:65], 1.0)

            qt_pair = [big_pool.tile([128, N], BF16, tag=f"qt{p}", name=f"qt{p}") for p in range(2)]
            kt_pair = [big_pool.tile([128, N], BF16, tag=f"kt{p}", name=f"kt{p}") for p in range(2)]
            ctx_pair = [big_pool.tile([128, N], BF16, tag=f"ctx{p}", name=f"ctx{p}") for p in range(2)]

            # ---- PE filler rounds (projections / V / out-projection) -------
            pe_ns = [0.0]  # emitted PE work (ns)
            act_ns = [0.0]  # emitted ACT work (ns)

            def qk_round(w8, pair, nq, dst):
                flush_evacs(keep=1)
                ps = ps_b.tile([128, 512], F32, tag="b", name="b")
                for e in range(8):
                    nc.tensor.matmul(
                        ps[:],
                        w8(e)[:, pair * 128:(pair + 1) * 128],
                        xt(e, nq),
                        start=(e == 0),
                        stop=(e == 7),
                    )
                deferred_evacs.append(
                    lambda: nc.vector.tensor_copy(
                        dst[pair][:, nq * 512:(nq + 1) * 512], ps[:]))
                pe_ns[0] += 8 * 512 / 2.4

            def v_round(nb):
                flush_evacs(keep=1)
                ps = ps_b.tile([128, 512], F32, tag="b", name="b")[:, 0:256]
                c, coff = divmod(nb, 4)
                for e in range(8):
                    nc.tensor.matmul(
                        ps[:],
                        xt(e, c)[:, coff * 128:(coff + 1) * 128],
                        wv8(e),
                        start=(e == 0),
                        stop=(e == 7),
                    )
                deferred_evacs.append(
                    lambda: nc.vector.tensor_copy(
                        v4[:, nb, :, 0:64],
                        ps[:].rearrange("p (h d) -> p h d", h=4)))
                pe_ns[0] += 8 * 256 / 2.4

            osb_tiles = {}

            COPY = mybir.ActivationFunctionType.Copy

            # Round evacuations are DEFERRED: each round emits its matmuls
            # immediately and queues the PSUM->SBUF evacuation, which is
            # flushed AFTER the next S^T chunk's causal-band ADD + exp are
            # emitted.  The DVE executes in order, so an evacuation emitted
            # ahead of the ADD would stall the exp chain that paces the PE.
            # ps_b has 2 bufs: a new round flushes all but the newest evac
            # before its matmuls so the bank it reuses has been drained.
            deferred_evacs = []

            def flush_evacs(keep=0):
                while len(deferred_evacs) > keep:
                    deferred_evacs.pop(0)()

            def o_round(nb, fc, act_evac=False):
                flush_evacs(keep=1)
                if fc == 0:
                    osb_tiles[nb] = osb_pool.tile([128, D], BF16, tag="osb", name="osb")
                osb = osb_tiles[nb]
                ps = ps_b.tile([128, 512], F32, tag="b", name="b")
                for pr2 in range(2):
                    nc.tensor.matmul(
                        ps[:],
                        ctx_pair[pr2][:, nb * 128:(nb + 1) * 128],
                        wo2[pr2][:, fc * 512:(fc + 1) * 512],
                        start=(pr2 == 0),
                        stop=(pr2 == 1),
                    )

                def evac(nb=nb, fc=fc, osb=osb, ps=ps, act_evac=act_evac):
                    if act_evac:
                        # Copy shares the exp ACT table (no swap); the tail's
                        # evacuations ride ScalarE so the DVE is free for the
                        # final normalize dance
                        nc.scalar.activation(
                            osb[:, fc * 512:(fc + 1) * 512], ps[:], COPY)
                    else:
                        nc.vector.tensor_copy(
                            osb[:, fc * 512:(fc + 1) * 512], ps[:])
                    nc.sync.dma_start(
                        out_d[nb * 128:(nb + 1) * 128, fc * 512:(fc + 1) * 512],
                        osb[:, fc * 512:(fc + 1) * 512],
                    )

                deferred_evacs.append(evac)
                if fc == 1:
                    del osb_tiles[nb]
                pe_ns[0] += 2 * 512 / 2.4

            rounds = {}
            for pair in range(2):
                for nq in range(NQT):
                    rounds[("q", pair, nq)] = (lambda p=pair, n=nq: qk_round(wq8, p, n, qt_pair))
                    rounds[("k", pair, nq)] = (lambda p=pair, n=nq: qk_round(wk8, p, n, kt_pair))
            for nb in range(kb_max):
                rounds[("v", nb)] = (lambda n=nb: v_round(n))
            for nb in range(NB):
                for fc in range(2):
                    rounds[("o", nb, fc)] = (lambda n=nb, f=fc: o_round(n, f))

            emitted = set()
            filler_q = []

            def emit_rid(rid):
                if rid in emitted:
                    return
                emitted.add(rid)
                rounds[rid]()

            def inject_fillers(headroom=4000.0):
                # keep ~4us of emitted-but-unexecuted PE work beyond the ACT
                # frontier so the PE (the critical engine) never drains
                while filler_q and pe_ns[0] < act_ns[0] + headroom:
                    emit_rid(filler_q.pop(0))

            # ---- normalize -------------------------------------------------
            # The DVE reciprocal is an iterative 8-cyc/element divide
            # streaming the FREE dim, and the denominator row is 512 elements
            # on ONE partition.  Use the DVE 32x32 StreamTranspose to fold
            # the row onto 32 partitions, take the reciprocal 16-wide, fold
            # back (bf16: ~0.4% rms on the normalize scale), partition-
            # broadcast on GpSimd, and multiply ctx (read straight from
            # PSUM) by the broadcast row.
            def norm_phase_a(pair, hh, qt, ctx_ps, c0, cw):
                """transpose/recip/transpose + gpsimd broadcast for columns
                [c0, c0+cw); returns the rbr tile for the multiply."""
                nblk = cw // 32
                tscat = work_pool.tile([32, 512], F32, tag="tscat", name="tscat")
                nc.vector.transpose(tscat[:, 0:cw], ctx_ps[64:96, c0:c0 + cw])
                rscat = work_pool.tile([32, 512], BF16, tag="rscat", name="rscat")
                with nc.allow_low_precision(
                    reason="bf16 softmax-denominator reciprocal: ~0.4% rms "
                    "on the normalize scale, inside the error budget"
                ):
                    nc.vector.reciprocal(
                        rscat[:, 0:cw].rearrange("p (b s) -> p b s", s=32)[:, :, 0],
                        tscat[:, 0:cw].rearrange("p (b s) -> p b s", s=32)[:, :, 0],
                    )
                rrow = work_pool.tile([32, 512], BF16, tag="rrow", name="rrow")
                nc.vector.transpose(rrow[:, 0:cw], rscat[:, 0:cw])
                act_ns[0] += 2 * (cw + 352) / 1.2
                rbr = work_pool.tile([64, 512], BF16, tag="rbr", name="rbr")
                # GpSimd runs ONLY partition_broadcast ops (+ the startup
                # memsets): op-type churn makes walrus swap the firmware
                # library (~7us per swap)
                nc.gpsimd.partition_broadcast(rbr[:, 0:cw], rrow[0:1, 0:cw])
                return rbr

            def norm_phase_b(pair, hh, qt, ctx_ps, rbr, c0, cw):
                hp = slice(64 * hh, 64 * hh + 64)
                nc.vector.tensor_mul(
                    ctx_pair[pair][hp, qt * 512 + c0:qt * 512 + c0 + cw],
                    ctx_ps[0:64, c0:c0 + cw],
                    rbr[:, 0:cw],
                )

            done_norms = {q: 0 for q in range(NQT)}

            def note_norm_done(nqt):
                done_norms[nqt] += 1
                if done_norms[nqt] == 2:
                    for nb in range(4 * nqt, 4 * nqt + 4):
                        filler_q.append(("o", nb, 0))
                        filler_q.append(("o", nb, 1))

            # ---- attention unit: S^T + exp stream with own PV riffled ------
            def emit_unit(pair, qt, nchunks, pending_muls):
                # the hard-emitted q/k/v prereq rounds' evacuations WRITE the
                # kt/qt/v4 tiles this stream reads -- they must be emitted
                # before any S^T/PV (the Tile tracker is emission-ordered)
                flush_evacs()
                ctx2 = [
                    ps_ctx.tile([96, 512], F32, tag=f"ctx{hh}", name=f"ctx{hh}")
                    for hh in range(2)
                ]
                pvq = []

                def pv_chunk(j, ptt, off):
                    for hh in range(2):
                        nc.tensor.matmul(
                            ctx2[hh][0:65, off:],
                            v4[:, j, 2 * pair + hh, :],
                            ptt[:, hh, off:],
                            start=(j == 0),
                            stop=(j == nchunks - 1),
                            skip_group_check=True,
                        )
                    pe_ns[0] += 2 * (512 - off) / 2.4

                for j in range(nchunks):
                    if j == 1 and pending_muls:
                        # prev unit's normalize multiplies, flushed after the
                        # gpsimd broadcasts have had ~2 chunks to complete so
                        # the in-order DVE queue never stalls on them
                        for fn in pending_muls:
                            fn()
                        pending_muls.clear()
                    while pvq and pvq[0][0] <= j - RIFFLE_LAG:
                        pv_chunk(*pvq.pop(0))
                    inject_fillers()
                    d = j - 4 * qt
                    # exact-causal column trim (keep matmul N >= 128)
                    off = 128 * d if d >= 1 else 0
                    st_ps = ps_st.tile([128, 2, 512], F32, tag="blk", name="blk")
                    # For diagonal chunks, compute the masked 128-col slab
                    # FIRST and emit the DVE causal add right after it: the
                    # add then executes UNDER the clean-slab matmuls instead
                    # of sitting serially between S^T and exp (that latency
                    # paces the whole exp stream).
                    w = min(128, 512 - off) if d >= 0 else 0
                    for hh in range(2):
                        hp = slice(64 * hh, 64 * hh + 64)
                        nc.tensor.matmul(
                            st_ps[:, hh, off:off + w] if d >= 0 else
                            st_ps[:, hh, off:],
                            kt_pair[pair][hp, j * 128:(j + 1) * 128],
                            qt_pair[pair][hp, qt * 512 + off:
                                          qt * 512 + off + w] if d >= 0 else
                            qt_pair[pair][hp, qt * 512 + off:(qt + 1) * 512],
                            start=True,
                            stop=True,
                        )
                    if d >= 0:
                        # causal add -30000; with off = 128*d the masked
                        # triangle lies entirely in cols [off, off+128);
                        # one op covers both heads via a stride-0 broadcast
                        u0 = 384 - 128 * d + off
                        nc.vector.tensor_tensor(
                            st_ps[:, :, off:off + w],
                            st_ps[:, :, off:off + w],
                            trineg_t[:, u0:u0 + w].unsqueeze(1).broadcast_to(
                                (128, 2, w)
                            ),
                            ADD,
                        )
                        if off + w < 512:
                            for hh in range(2):
                                hp = slice(64 * hh, 64 * hh + 64)
                                nc.tensor.matmul(
                                    st_ps[:, hh, off + w:],
                                    kt_pair[pair][hp, j * 128:(j + 1) * 128],
                                    qt_pair[pair][hp, qt * 512 + off + w:
                                                  (qt + 1) * 512],
                                    start=True,
                                    stop=True,
                                )
                    pe_ns[0] += (512 - off) / 2.4
                    pt_t = pt_pool.tile([128, 2, 512], BF16, tag="pt", name="pt")
                    kw = {}
                    if j >= jpad_min:  # per-key pad bias (same for both heads)
                        kw["bias"] = padb_t[:, j:j + 1]
                    nc.scalar.activation(
                        pt_t[:, :, off:], st_ps[:, :, off:], EXP, scale=SCALE, **kw
                    )
                    act_ns[0] += (2 * (512 - off) + 352) / 1.2
                    flush_evacs(keep=1)
                    pvq.append((j, pt_t, off))
                if pending_muls:
                    for fn in pending_muls:
                        fn()
                    pending_muls.clear()
                while pvq:
                    pv_chunk(*pvq.pop(0))
                return ctx2

            # Unit order: start with the cheapest unit (least input DMA),
            # end with a 4-chunk unit so the post-last-exp tail (PV drain +
            # normalize + o-rounds + out DMA) is short.
            units = [(0, 0), (0, 1), (1, 1), (0, 3), (1, 3), (0, 2), (1, 0), (1, 2)]
            units = [(p, qt, min(4 * qt + 4, kb_max)) for (p, qt) in units]

            # projection/V rounds become filler, ordered by the deadline of
            # the unit that first needs them (emit_rid dedups, so rounds the
            # unit loop hard-emits are simply skipped here)
            seen_rounds = set()
            for (p, qt, nch) in units:
                for rid in ([("q", p, qt)]
                            + [("k", p, nq) for nq in range(qt + 1)]
                            + [("v", nb) for nb in range(nch)]):
                    if rid not in seen_rounds:
                        seen_rounds.add(rid)
                        filler_q.append(rid)

            pending_muls = []
            prev = None  # (pair, qt, ctx2)
            for idx, (pair, qt, nchunks) in enumerate(units):
                # HARD-emit this unit's projection/V prereqs before any of
                # its S^T/PV instructions.  A read emitted before its writer
                # gets NO dependency from the Tile tracker (emission-ordered)
                # and would consume uninitialized SBUF; filler pacing alone
                # must never be trusted for correctness.
                emit_rid(("q", pair, qt))
                for nq in range(qt + 1):
                    emit_rid(("k", pair, nq))
                for nb in range(nchunks):
                    emit_rid(("v", nb))
                if prev is not None:
                    # previous unit's normalize: dance+broadcast now, the
                    # multiplies 2 chunks into this unit's stream.  This MUST
                    # precede emit_unit: ps_ctx has bufs=1, so this unit's PV
                    # start-write reuses the previous ctx banks and the Tile
                    # tracker (emission-ordered) only serializes them if the
                    # normalize reads are emitted first.
                    ppair, pqt, pctx2 = prev
                    for hh in range(2):
                        rbr = norm_phase_a(ppair, hh, pqt, pctx2[hh], 0, 512)
                        pending_muls.append(
                            (lambda p=ppair, h=hh, q=pqt, c=pctx2[hh], r=rbr:
                             norm_phase_b(p, h, q, c, r, 0, 512))
                        )
                    pending_muls.append(lambda q=pqt: note_norm_done(q))
                ctx2 = emit_unit(pair, qt, nchunks, pending_muls)
                prev = (pair, qt, ctx2)

            # ---- tail: last unit's normalize in halves + o-rounds ----------
            # (any leftover dep-ready fillers first, so they precede the
            # normalize chain in each engine's in-order queue)
            flush_evacs()
            lpair, lqt, lctx2 = prev
            # all four dance pieces first (DVE runs them back-to-back while
            # GpSimd broadcasts trail); any leftover filler rounds are
            # emitted NEXT so the PE has work during the dance; then per
            # half: multiplies + o-rounds with ScalarE evacuations
            rbrs = {}
            for c0 in (0, 256):
                for hh in range(2):
                    rbrs[(c0, hh)] = norm_phase_a(
                        lpair, hh, lqt, lctx2[hh], c0, 256)
            while filler_q:
                emit_rid(filler_q.pop(0))
            flush_evacs()
            for c0 in (0, 256):
                for hh in range(2):
                    norm_phase_b(lpair, hh, lqt, lctx2[hh], rbrs[(c0, hh)],
                                 c0, 256)
                for nb in range(4 * lqt + c0 // 128, 4 * lqt + c0 // 128 + 2):
                    for fc in range(2):
                        emitted.add(("o", nb, fc))
                        o_round(nb, fc, act_evac=True)
            flush_evacs()

    nc.compile()
    return nc


_PROGRAM_CACHE = {}


def kernel(x, attention_mask, W_Q, W_K, W_V, W_out, b_out):
    global LAST_RESULTS
    from concourse.bass_utils import run_bass_kernel_spmd

    x = np.ascontiguousarray(x, dtype=np.float32)
    attention_mask = np.asarray(attention_mask)
    lengths = attention_mask.astype(np.int64).sum(axis=1)
    kb_max = int(math.ceil(lengths.max() / KBLK))
    jpad_min = int(lengths.min() // KBLK)

    key = (kb_max, jpad_min)
    if key not in _PROGRAM_CACHE:
        _PROGRAM_CACHE[key] = _build_program(kb_max, jpad_min)
    nc = _PROGRAM_CACHE[key]

    # host-side input prep (matmul operands pre-cast to bf16)
    import ml_dtypes
    BF = ml_dtypes.bfloat16
    xT = [np.ascontiguousarray(x[b].T.astype(BF)) for b in range(B)]
    wqT = np.ascontiguousarray(np.asarray(W_Q, dtype=np.float32).T.astype(BF))
    wkT = np.ascontiguousarray(np.asarray(W_K, dtype=np.float32).T.astype(BF))
    wvT = np.ascontiguousarray(np.asarray(W_V, dtype=np.float32).T.astype(BF))
    woT = np.ascontiguousarray(np.asarray(W_out, dtype=np.float32).T.astype(BF))
    # padbias[p, j] = 0 if key j*128+p is real else NEGB
    padb = [
        np.ascontiguousarray(
            np.where(attention_mask[b].reshape(16, 128).T != 0, 0.0, NEGB)
        ).astype(np.float32)
        for b in range(B)
    ]
    # trineg[p, u] = NEG if u < p + 384 else 0; slice [384-128d : 896-128d]
    # gives the causal additive mask for a diagonal block with offset 128d.
    pp = np.arange(128)[:, None]
    uu = np.arange(896)[None, :]
    trineg = np.where(uu < pp + 384, NEG, 0.0).astype(BF)

    in_maps = []
    for c in range(NCORES):
        b, g = divmod(c, 4)
        sl = slice(g * 256, (g + 1) * 256)
        in_maps.append(
            {
                "xt": xT[b],
                "wq": np.ascontiguousarray(wqT[:, sl]),
                "wk": np.ascontiguousarray(wkT[:, sl]),
                "wv": np.ascontiguousarray(wvT[:, sl]),
                "wout": np.ascontiguousarray(woT[sl, :]),
                "padbias": padb[b],
                "trineg": trineg,
            }
        )

    trace = bool(int(os.environ.get("KERNEL_TRACE", "0")))
    ncores_run = int(os.environ.get("KERNEL_NCORES", str(NCORES)))
    res = run_bass_kernel_spmd(
        nc,
        in_maps[:ncores_run],
        core_ids=list(range(ncores_run)),
        trace=trace,
        trace_cores=list(range(ncores_run)) if trace else None,
    )
    LAST_RESULTS = res

    out = np.zeros((B, N, D), dtype=np.float32)
    for c in range(len(res.results)):
        out[c // 4] += np.asarray(res.results[c]["out"], dtype=np.float32)
    out += np.asarray(b_out, dtype=np.float32)[None, None, :]
    return out


# revision 12
# speedup vs baseline: 1.1019x; 1.1019x over previous
"""Trainium2 Bass kernel for causal+padded multi-head attention.

Problem: B=2, N=2048, D=1024, H=16 heads (DK=64), fp32 I/O.
  out = softmax(mask(x Wq^T (x Wk^T)^T) / sqrt(DK)) (x Wv^T) Wout^T + b_out

Sharding (8 cores): core c handles batch b=c//4 and heads [4*(c%4), 4*(c%4)+4).
Each core computes a partial output [N, D] (its 4 heads' contribution through
the output projection, bf16); the host sums the 4 partials per batch in fp32
and adds b_out.

Engine assignment per core (~156us, vs 222us for the phase-serial baseline):
  PE     all matmuls: QKV projection rounds, S^T (row-tiled head pairs),
         PV (with a ones column appended to V so the same matmul yields the
         softmax denominators), out-projection rounds (~120us busy)
  ACT    softmax exp only: exp(0.125*s + pad_bias) -> bf16 (~75us)
  DVE    PSUM evacuations (casts), causal band adds, denominator reciprocal
  GPSIMD denominator partition-broadcast ONLY (mixing gpsimd op types makes
         walrus swap the firmware library around every op, ~7us each)
  DMA    both queues (SP + ACT) for the input load; bf16 outputs

Schedule: attention units (head-pair, q-tile 512) run in sequence; a unit's
S^T->exp chain is ACT-bound, so projection/V/out-projection rounds are
injected between S^T matmuls as PE "filler" to keep the PE dense (HAM stays
at K=8/8).  Each unit's prereq rounds are HARD-emitted before the unit: the
Tile tracker is emission-ordered, so a read emitted before its writer gets no
dependency and reads garbage — filler pacing is a performance heuristic only.
PV matmuls of unit k are riffled into unit k+1's S^T stream with a small lag
so the in-order PE never drains on ScalarE.  Normalization runs one unit
behind: the denominator row [1,512] is folded onto 32 partitions with the
DVE 32x32 StreamTranspose, reciprocal'd 16-wide (the iterative divide is
8 cyc/element along the free dim), folded back, partition-broadcast on
GpSimd, and multiplied into ctx on DVE.  The [B,H,N,N] score tensor is
causally trimmed at 128-column granularity in S^T, exp, and PV.

Known dead ends (measured): fp8 anywhere gives 4-8e-2 rel err (gate 1e-2);
reciprocal_approx_fast (custom DVE uop) returns garbage/crashes the exec unit
under this runtime; exp(-ln(den)) on ScalarE forces an ACT table swap per
call (~38us total); SBUF->SBUF DMA cannot cross partitions, and DRAM-bounce
transposes race (DMA queue issue is async, Tile does not serialize the DRAM
RAW).
"""

import math
import os

import numpy as np

B, N, D, H = 2, 2048, 1024, 16
DK = D // H  # 64
NCORES = 8
HEADS_PER_CORE = 4
QTILE = 512
KBLK = 128
NEG = -30000.0
NEGB = -3750.0  # pad bias applied after the 0.125 scale inside exp
SCALE = 1.0 / math.sqrt(float(DK))  # 0.125
RIFFLE_LAG = 2  # PV chunks of unit k trail unit k+1's S^T by this many j's

# Set by run() when tracing is enabled (test.py reads this).
LAST_RESULTS = None


def _build_program(kb_max: int, jpad_min: int):
    import concourse.tile as tile
    from concourse import bacc, mybir

    F32 = mybir.dt.float32
    BF16 = mybir.dt.bfloat16
    EXP = mybir.ActivationFunctionType.Exp
    ADD = mybir.AluOpType.add

    nc = bacc.Bacc(None)

    xt_d = nc.dram_tensor("xt", [D, N], BF16, kind="ExternalInput")
    wq_d = nc.dram_tensor("wq", [D, 256], BF16, kind="ExternalInput")
    wk_d = nc.dram_tensor("wk", [D, 256], BF16, kind="ExternalInput")
    wv_d = nc.dram_tensor("wv", [D, 256], BF16, kind="ExternalInput")
    wout_d = nc.dram_tensor("wout", [256, D], BF16, kind="ExternalInput")
    padb_d = nc.dram_tensor("padbias", [128, 16], F32, kind="ExternalInput")
    trineg_d = nc.dram_tensor("trineg", [128, 896], BF16, kind="ExternalInput")
    out_d = nc.dram_tensor("out", [N, D], BF16, kind="ExternalOutput")

    NB = N // KBLK  # 16 key/row blocks
    NQT = N // QTILE  # 4 q tiles

    with tile.TileContext(nc) as tc:
        with (
            tc.tile_pool(name="w", bufs=1) as w_pool,
            tc.tile_pool(name="big", bufs=1) as big_pool,
            tc.tile_pool(name="work", bufs=4) as work_pool,
            tc.tile_pool(name="osb", bufs=3) as osb_pool,
            tc.tile_pool(name="xt", bufs=1) as xt_pool,
            tc.tile_pool(name="pt", bufs=26) as pt_pool,
            tc.tile_pool(name="ps_st", bufs=2, space="PSUM") as ps_st,
            tc.tile_pool(name="ps_b", bufs=2, space="PSUM") as ps_b,
            tc.tile_pool(name="ps_ctx", bufs=1, space="PSUM") as ps_ctx,
        ):
            # PE clock warm-up: TRN2 throttles the PE to 1.2/0.65 GHz
            # after idle and needs ~3-5us of continuous execution (4us HAM
            # windows) to reach 2.4 GHz.  A dense chain of matmuls on a
            # memset tile runs during the DMA head so the first real rounds
            # execute at full clock instead of ramping through them.
            warmw = w_pool.tile([128, 512], BF16, tag="warmw", name="warmw")
            nc.vector.memset(warmw[:], 0.03)
            wps = ps_b.tile([128, 512], F32, tag="b", name="b")
            for _ in range(15):
                nc.tensor.matmul(wps[:], warmw[:, 0:128], warmw[:],
                                 start=True, stop=True)

            # ---- input DMAs (weights first so projections can start early) --
            wq_t = w_pool.tile([128, 8, 256], BF16, tag="wq")
            wk_t = w_pool.tile([128, 8, 256], BF16, tag="wk")
            wv_t = w_pool.tile([128, 8, 256], BF16, tag="wv")
            wo_t = w_pool.tile([128, 2, D], BF16, tag="wo")
            padb_t = w_pool.tile([128, 16], F32, tag="padb")
            trineg_t = w_pool.tile([128, 896], BF16, tag="trineg")
            # Input DMAs ride BOTH hardware DMA queues (SP + Activation) —
            # a single queue is strictly serial at ~220GB/s and the load
            # gates the whole pipeline.  xt arrives as 32 column-chunked
            # tiles, q-tile-major, so the first projection rounds complete
            # after ~1MB instead of the full 4MB load.
            # Input DMAs ride BOTH hardware DMA queues (SP + Activation) —
            # a single queue is strictly serial at ~220GB/s and the load
            # gates the whole pipeline.  xt arrives as 32 column-chunked
            # tiles, q-tile-major, so the first projection rounds complete
            # after ~1MB instead of the full 4MB load.
            nc.sync.dma_start(wq_t[:], wq_d[:].rearrange("(e p) m -> p e m", p=128))
            nc.scalar.dma_start(wk_t[:], wk_d[:].rearrange("(e p) m -> p e m", p=128))
            xt = [[None] * NQT for _ in range(8)]
            for c in range(NQT):
                for e in range(8):
                    t = xt_pool.tile(
                        [128, 512], BF16, tag=f"xt{e}_{c}", name=f"xt{e}_{c}"
                    )
                    # ACT-queue DMAs only for the head-critical c=0 chunks;
                    # later chunks stay off the exp-stream queue
                    eng = nc.scalar if (c == 0 and e >= 4) else nc.sync
                    eng.dma_start(
                        t[:], xt_d[e * 128:(e + 1) * 128, c * 512:(c + 1) * 512]
                    )
                    xt[e][c] = t
                if c == 0:
                    nc.sync.dma_start(wv_t[:], wv_d[:].rearrange("(e p) m -> p e m", p=128))
                    nc.scalar.dma_start(trineg_t[:], trineg_d[:])
                    nc.scalar.dma_start(padb_t[:], padb_d[:])
            nc.sync.dma_start(wo_t[:], wout_d[:].rearrange("(c p) m -> p c m", p=128))

            # V' tile: [keys 128, key-block, head 4, 65]; col 64 <- ones so
            # P@V' also yields the softmax denominator on ctx row 64.
            v4 = big_pool.tile([128, kb_max, 4, 65], BF16, tag="v4")
            nc.gpsimd.memset(v4[:, :, :, 64:65], 1.0)

            # warm the ACT exp table during the DMA head so the ~2.7us
            # table load is off the critical path
            warm = work_pool.tile([1, 8], F32, tag="warm", name="warm")
            nc.vector.memset(warm[:], 1.0)
            nc.scalar.activation(warm[:], warm[:], EXP)

            qt_pair = [big_pool.tile([128, N], BF16, tag=f"qt{p}", name=f"qt{p}") for p in range(2)]
            kt_pair = [big_pool.tile([128, N], BF16, tag=f"kt{p}", name=f"kt{p}") for p in range(2)]
            ctx_pair = [big_pool.tile([128, N], BF16, tag=f"ctx{p}", name=f"ctx{p}") for p in range(2)]

            # ---- PE filler rounds (projections / V / out-projection) -------
            # Each round is ~1-4us of dense PE work ending in one DVE
            # evacuation; they are injected between attention steps to keep
            # the PE busy while ScalarE works through the exp chain.
            pe_ns = [0.0]  # emitted PE work (ns)
            act_ns = [0.0]  # emitted ACT work (ns)

            def qk_round(w_t, pair, nq, dst):
                ps = ps_b.tile([128, 512], F32, tag="b", name="b")
                for e in range(8):
                    nc.tensor.matmul(
                        ps[:],
                        w_t[:, e, pair * 128:(pair + 1) * 128],
                        xt[e][nq][:],
                        start=(e == 0),
                        stop=(e == 7),
                    )
                nc.vector.tensor_copy(dst[pair][:, nq * 512:(nq + 1) * 512], ps[:])
                pe_ns[0] += 8 * 512 / 2.4

            def v_round(nb):
                ps = ps_b.tile([128, 512], F32, tag="b", name="b")[:, 0:256]
                c, coff = divmod(nb, 4)
                for e in range(8):
                    nc.tensor.matmul(
                        ps[:],
                        xt[e][c][:, coff * 128:(coff + 1) * 128],
                        wv_t[:, e, :],
                        start=(e == 0),
                        stop=(e == 7),
                    )
                nc.vector.tensor_copy(
                    v4[:, nb, :, 0:64], ps[:].rearrange("p (h d) -> p h d", h=4)
                )
                pe_ns[0] += 8 * 256 / 2.4

            osb_tiles = {}

            def o_round(nb, fc, tail=False):
                if fc == 0:
                    osb_tiles[nb] = osb_pool.tile([128, D], BF16, tag="osb", name="osb")
                osb = osb_tiles[nb]
                ps = ps_b.tile([128, 512], F32, tag="b", name="b")
                for pr2 in range(2):
                    nc.tensor.matmul(
                        ps[:],
                        ctx_pair[pr2][:, nb * 128:(nb + 1) * 128],
                        wo_t[:, pr2, fc * 512:(fc + 1) * 512],
                        start=(pr2 == 0),
                        stop=(pr2 == 1),
                    )
                if tail:
                    # Copy shares the exp ACT table (no swap): the final
                    # rounds evacuate on ScalarE (idle after the last exp)
                    # so the DVE is free for the last normalize dance, and
                    # DMA per half so the final drain is shorter
                    nc.scalar.activation(
                        osb[:, fc * 512:(fc + 1) * 512], ps[:],
                        mybir.ActivationFunctionType.Copy)
                    nc.sync.dma_start(
                        out_d[nb * 128:(nb + 1) * 128, fc * 512:(fc + 1) * 512],
                        osb[:, fc * 512:(fc + 1) * 512])
                    if fc == 1:
                        del osb_tiles[nb]
                else:
                    nc.vector.tensor_copy(osb[:, fc * 512:(fc + 1) * 512], ps[:])
                    if fc == 1:
                        nc.sync.dma_start(out_d[nb * 128:(nb + 1) * 128, :], osb[:])
                        del osb_tiles[nb]
                pe_ns[0] += 2 * 512 / 2.4

            rounds = {}
            for pair in range(2):
                for nq in range(NQT):
                    rounds[("q", pair, nq)] = (lambda p=pair, n=nq: qk_round(wq_t, p, n, qt_pair))
                    rounds[("k", pair, nq)] = (lambda p=pair, n=nq: qk_round(wk_t, p, n, kt_pair))
            for nb in range(kb_max):
                rounds[("v", nb)] = (lambda n=nb: v_round(n))
            for nb in range(NB):
                for fc in range(2):
                    rounds[("o", nb, fc)] = (lambda n=nb, f=fc: o_round(n, f))

            emitted = set()
            filler_q = []

            def emit_rid(rid):
                if rid in emitted:
                    return
                emitted.add(rid)
                rounds[rid]()

            def inject_fillers():
                # keep ~4us of emitted-but-unexecuted PE work beyond the ACT
                # frontier so the PE (the critical engine) never drains
                while filler_q and pe_ns[0] < act_ns[0] + 4000.0:
                    emit_rid(filler_q.pop(0))

            # prereqs of the first two units, emitted up front
            for rid in [("q", 0, 0), ("k", 0, 0), ("q", 1, 0), ("k", 1, 0),
                        ("v", 0), ("v", 1), ("v", 2), ("v", 3)]:
                emit_rid(rid)
            # remaining projection/V rounds become filler, ordered by the
            # deadline of the unit that first needs them
            for qt in range(1, NQT):
                for pair in range(2):
                    filler_q.append(("q", pair, qt))
                    filler_q.append(("k", pair, qt))
                for nb in range(4 * qt, min(4 * qt + 4, kb_max)):
                    filler_q.append(("v", nb))

            # ---- attention units ------------------------------------------
            def emit_normalize(pair, hh, qt, ctx_ps):
                # The DVE reciprocal is an iterative 8-cyc/element divide
                # streaming the FREE dim, and the denominator row is 512
                # elements on ONE partition (3.3us/call there).  Use the
                # DVE 32x32 StreamTranspose to fold the row onto 32
                # partitions, take the reciprocal 16-wide, and fold back
                # (bf16 on the way back: DVE 2x, ~0.4% rms).
                hp = slice(64 * hh, 64 * hh + 64)
                craw = work_pool.tile([65, 512], F32, tag="craw", name="craw")
                nc.vector.tensor_copy(craw[:], ctx_ps[0:65, :])
                tscat = work_pool.tile([32, 512], F32, tag="tscat", name="tscat")
                nc.vector.transpose(tscat[:], ctx_ps[64:96, :])
                rscat = work_pool.tile([32, 512], BF16, tag="rscat", name="rscat")
                with nc.allow_low_precision(
                    reason="bf16 softmax-denominator reciprocal: ~0.4% rms "
                    "on the normalize scale, inside the error budget"
                ):
                    nc.vector.reciprocal(
                        rscat[:].rearrange("p (b s) -> p b s", s=32)[:, :, 0],
                        tscat[:].rearrange("p (b s) -> p b s", s=32)[:, :, 0],
                    )
                rrow = work_pool.tile([32, 512], BF16, tag="rrow", name="rrow")
                nc.vector.transpose(rrow[:], rscat[:])
                act_ns[0] += 2 * (512 + 352) / 1.2
                rbr = work_pool.tile([64, 512], BF16, tag="rbr", name="rbr")
                # GpSimd runs ONLY partition_broadcast ops: mixing op types
                # makes walrus swap the gpsimd firmware library around every
                # op (~7us per swap)
                nc.gpsimd.partition_broadcast(rbr[:], rrow[0:1, :])
                nc.vector.tensor_mul(
                    ctx_pair[pair][hp, qt * 512:(qt + 1) * 512],
                    craw[0:64, :],
                    rbr[:],
                )

            last_unit = [False]  # force-drain fillers during the final unit

            def emit_st_exp(pair, qt, nchunks, prev):
                """S^T + mask + exp for both heads, with the previous unit's
                PV matmuls riffled in (lagged so the PE never stalls on the
                exp pipeline) and PE filler rounds injected to cover the
                ACT-bound stretch.  Returns PV descriptors."""
                if prev is None:
                    ppv = []
                else:
                    ppair, pqt, pn, ppv, pctx2 = prev

                def rif(k):
                    while ppv and ppv[0][0] <= k:
                        jj, ptt, poff = ppv.pop(0)
                        for hh in range(2):
                            nc.tensor.matmul(
                                pctx2[hh][0:65, poff:],
                                v4[:, jj, 2 * ppair + hh, :],
                                ptt[:, hh, poff:],
                                start=(jj == 0),
                                stop=(jj == pn - 1),
                                skip_group_check=True,
                            )
                        pe_ns[0] += 2 * (512 - poff) / 2.4

                pv = []
                for j in range(nchunks):
                    rif(j - RIFFLE_LAG)
                    inject_fillers()
                    if last_unit[0]:
                        # the PE FIFO can't reach past the last unit's
                        # exp-paced S^T stream; spread the remaining
                        # dep-ready fillers through it instead of after it
                        for _ in range(2):
                            if filler_q:
                                emit_rid(filler_q.pop(0))
                    d = j - 4 * qt
                    # exact-causal column trim (keep matmul N >= 128)
                    off = 128 * d if d >= 1 else 0
                    st_ps = ps_st.tile([128, 2, 512], F32, tag="blk", name="blk")
                    for hh in range(2):
                        hp = slice(64 * hh, 64 * hh + 64)
                        nc.tensor.matmul(
                            st_ps[:, hh, off:],
                            kt_pair[pair][hp, j * 128:(j + 1) * 128],
                            qt_pair[pair][hp, qt * 512 + off:(qt + 1) * 512],
                            start=True,
                            stop=True,
                        )
                    pe_ns[0] += (512 - off) / 2.4
                    if d >= 0:
                        # causal add -30000; with off = 128*d the masked
                        # triangle lies entirely in cols [off, off+128);
                        # one op covers both heads via a stride-0 broadcast
                        u0 = 384 - 128 * d + off
                        w = min(128, 512 - off)
                        nc.vector.tensor_tensor(
                            st_ps[:, :, off:off + w],
                            st_ps[:, :, off:off + w],
                            trineg_t[:, u0:u0 + w].unsqueeze(1).broadcast_to(
                                (128, 2, w)
                            ),
                            ADD,
                        )
                    pt_t = pt_pool.tile([128, 2, 512], BF16, tag="pt")
                    kw = {}
                    if j >= jpad_min:  # per-key pad bias (same for both heads)
                        kw["bias"] = padb_t[:, j:j + 1]
                    nc.scalar.activation(
                        pt_t[:, :, off:], st_ps[:, :, off:], EXP, scale=SCALE, **kw
                    )
                    act_ns[0] += (2 * (512 - off) + 352) / 1.2
                    pv.append((j, pt_t, off))
                rif(10 ** 9)
                return pv

            def emit_pv(pair, qt, nchunks, pv, ctx2):
                for j, pt_t, off in pv:
                    for hh in range(2):
                        nc.tensor.matmul(
                            ctx2[hh][0:65, off:],
                            v4[:, j, 2 * pair + hh, :],
                            pt_t[:, hh, off:],
                            start=(j == 0),
                            stop=(j == nchunks - 1),
                            skip_group_check=True,
                        )
                    pe_ns[0] += 2 * (512 - off) / 2.4

            units = [
                (pair, qt, min(4 * qt + 4, kb_max))
                for qt in range(NQT)
                for pair in range(2)
            ]
            done_norms = {q: 0 for q in range(NQT)}

            def emit_norm_unit(npair, nqt, nctx2):
                for hh in range(2):
                    emit_normalize(npair, hh, nqt, nctx2[hh])
                done_norms[nqt] += 1
                if done_norms[nqt] == 2:
                    # out-projection rounds for this q-tile become filler
                    for nb in range(4 * nqt, 4 * nqt + 4):
                        filler_q.append(("o", nb, 0))
                        filler_q.append(("o", nb, 1))

            prev_pv = None  # (pair, qt, nchunks, pv_descs, ctx2)
            for pair, qt, nchunks in units:
                # HARD-emit this unit's projection/V prereqs before any of
                # its S^T/PV instructions.  A read emitted before its writer
                # gets NO dependency from the Tile tracker (emission-ordered)
                # and would consume uninitialized SBUF; filler pacing alone
                # must never be trusted for correctness.
                emit_rid(("q", pair, qt))
                for nq in range(qt + 1):
                    emit_rid(("k", pair, nq))
                for nb in range(nchunks):
                    emit_rid(("v", nb))
                last_unit[0] = (pair, qt, nchunks) == units[-1]
                pv = emit_st_exp(pair, qt, nchunks, prev_pv)
                if prev_pv is not None:
                    ppair, pqt, pn, ppv, pctx2 = prev_pv
                    emit_norm_unit(ppair, pqt, pctx2)
                # [96, 512] so the normalize can StreamTranspose rows 64:96
                # (only 0:65 are written; same single PSUM bank either way)
                ctx2 = [
                    ps_ctx.tile([96, 512], F32, tag=f"ctx{hh}", name=f"ctx{hh}")
                    for hh in range(2)
                ]
                prev_pv = (pair, qt, nchunks, pv, ctx2)
            ppair, pqt, pn, ppv, pctx2 = prev_pv
            emit_pv(ppair, pqt, pn, ppv, pctx2)

            # ---- tail: the last unit's normalize in 256-col halves with
            # the final out-projection rounds interleaved.  The dance pieces
            # go first (DVE back-to-back, GpSimd broadcasts trailing), reads
            # straight from PSUM (no craw copy); each half's multiplies then
            # release two o-rounds whose evacuations ride ScalarE.
            def tail_phase_a(hh, ctx_ps, c0, cw):
                tscat = work_pool.tile([32, 512], F32, tag="tscat", name="tscat")
                nc.vector.transpose(tscat[:, 0:cw], ctx_ps[64:96, c0:c0 + cw])
                rscat = work_pool.tile([32, 512], BF16, tag="rscat", name="rscat")
                with nc.allow_low_precision(
                    reason="bf16 softmax-denominator reciprocal: ~0.4% rms "
                    "on the normalize scale, inside the error budget"
                ):
                    nc.vector.reciprocal(
                        rscat[:, 0:cw].rearrange("p (b s) -> p b s", s=32)[:, :, 0],
                        tscat[:, 0:cw].rearrange("p (b s) -> p b s", s=32)[:, :, 0],
                    )
                rrow = work_pool.tile([32, 512], BF16, tag="rrow", name="rrow")
                nc.vector.transpose(rrow[:, 0:cw], rscat[:, 0:cw])
                rbr = work_pool.tile([64, 512], BF16, tag="rbr", name="rbr")
                nc.gpsimd.partition_broadcast(rbr[:, 0:cw], rrow[0:1, 0:cw])
                return rbr

            rbrs = {}
            for c0 in (0, 256):
                for hh in range(2):
                    rbrs[(c0, hh)] = tail_phase_a(hh, pctx2[hh], c0, 256)
            while filler_q:
                emit_rid(filler_q.pop(0))
            for c0 in (0, 256):
                for hh in range(2):
                    hp = slice(64 * hh, 64 * hh + 64)
                    nc.vector.tensor_mul(
                        ctx_pair[ppair][hp, pqt * 512 + c0:pqt * 512 + c0 + 256],
                        pctx2[hh][0:64, c0:c0 + 256],
                        rbrs[(c0, hh)][:, 0:256],
                    )
                for nb in range(4 * pqt + c0 // 128, 4 * pqt + c0 // 128 + 2):
                    for fc in range(2):
                        emitted.add(("o", nb, fc))
                        o_round(nb, fc, tail=True)

    nc.compile()
    return nc


_PROGRAM_CACHE = {}


def kernel(x, attention_mask, W_Q, W_K, W_V, W_out, b_out):
    global LAST_RESULTS
    from concourse.bass_utils import run_bass_kernel_spmd

    x = np.ascontiguousarray(x, dtype=np.float32)
    attention_mask = np.asarray(attention_mask)
    lengths = attention_mask.astype(np.int64).sum(axis=1)
    kb_max = int(math.ceil(lengths.max() / KBLK))
    jpad_min = int(lengths.min() // KBLK)

    key = (kb_max, jpad_min)
    if key not in _PROGRAM_CACHE:
        _PROGRAM_CACHE[key] = _build_program(kb_max, jpad_min)
    nc = _PROGRAM_CACHE[key]

    # host-side input prep (matmul operands pre-cast to bf16)
    import ml_dtypes
    BF = ml_dtypes.bfloat16
    xT = [np.ascontiguousarray(x[b].T.astype(BF)) for b in range(B)]
    wqT = np.ascontiguousarray(np.asarray(W_Q, dtype=np.float32).T.astype(BF))
    wkT = np.ascontiguousarray(np.asarray(W_K, dtype=np.float32).T.astype(BF))
    wvT = np.ascontiguousarray(np.asarray(W_V, dtype=np.float32).T.astype(BF))
    woT = np.ascontiguousarray(np.asarray(W_out, dtype=np.float32).T.astype(BF))
    # padbias[p, j] = 0 if key j*128+p is real else NEGB
    padb = [
        np.ascontiguousarray(
            np.where(attention_mask[b].reshape(16, 128).T != 0, 0.0, NEGB)
        ).astype(np.float32)
        for b in range(B)
    ]
    # trineg[p, u] = NEG if u < p + 384 else 0; slice [384-128d : 896-128d]
    # gives the causal additive mask for a diagonal block with offset 128d.
    pp = np.arange(128)[:, None]
    uu = np.arange(896)[None, :]
    trineg = np.where(uu < pp + 384, NEG, 0.0).astype(BF)

    in_maps = []
    for c in range(NCORES):
        b, g = divmod(c, 4)
        sl = slice(g * 256, (g + 1) * 256)
        in_maps.append(
            {
                "xt": xT[b],
                "wq": np.ascontiguousarray(wqT[:, sl]),
                "wk": np.ascontiguousarray(wkT[:, sl]),
                "wv": np.ascontiguousarray(wvT[:, sl]),
                "wout": np.ascontiguousarray(woT[sl, :]),
                "padbias": padb[b],
                "trineg": trineg,
            }
        )

    trace = bool(int(os.environ.get("KERNEL_TRACE", "0")))
    ncores_run = int(os.environ.get("KERNEL_NCORES", str(NCORES)))
    res = run_bass_kernel_spmd(
        nc,
        in_maps[:ncores_run],
        core_ids=list(range(ncores_run)),
        trace=trace,
        trace_cores=list(range(ncores_run)) if trace else None,
    )
    LAST_RESULTS = res

    out = np.zeros((B, N, D), dtype=np.float32)
    for c in range(len(res.results)):
        out[c // 4] += np.asarray(res.results[c]["out"], dtype=np.float32)
    out += np.asarray(b_out, dtype=np.float32)[None, None, :]
    return out

